# revision 6
# baseline (speedup 1.0000x reference)
"""MoD (Mixture-of-Depths) layer on 8 Trainium2 NeuronCores.

Strategy (hardcoded for B=4, N=4096, D=1024, H=16, C=1024):
  - Launch 1 (device): router scores s = tokens @ w_router for all B*N tokens.
    Sharded 8 ways: core c scores half-batch (b = c//2, half = c%2), 2048 tokens.
  - Host: top-C=1024 token selection per batch (set semantics - attention over the
    capacity buffer plus the scatter-back make the result order-invariant),
    z_loss = mean(scores^2), gate = sigmoid(selected scores).
  - Launch 2 (device): gather selected tokens (indirect DMA), multi-head
    self-attention over the C-token buffer, output projection, gating.
    Sharded 8 ways: core c handles batch c//2 and 8 of 16 heads (half c%2),
    producing a partial [C, D] output (partial over the wo contraction).
  - Host: sum core pairs, scatter rows into zeros([B, N, D]).
"""
import sys, types, functools

for _p in ('/opt/trn_rl_repo',):
    if _p not in sys.path:
        sys.path.append(_p)

import numpy as np
import ml_dtypes

import concourse.bacc as bacc
import concourse.mybir as mybir
import concourse.tile as tile
import concourse.bass as bass
from concourse.bass_utils import run_bass_kernel_spmd

BF16 = np.dtype(ml_dtypes.bfloat16)
P = 128
B, N, D, H, DH = 4, 4096, 1024, 16, 64
C = 1024            # expert capacity = N/4
NHALF = N // 2      # tokens per core in launch 1
HHALF = 512         # head-dim half (8 heads) per core in launch 2
NT1 = NHALF // P    # 16 score tiles per core
CT = C // P         # 8 capacity tiles
DT = D // P         # 8 feature tiles
KT = HHALF // P     # 4 tiles of the per-core head block


def _install_ntff_hook():
    """The image's antenv lacks axon_hooks; recreate it so trace=True works."""
    try:
        from antenv import axon_hooks  # noqa: F401
        return
    except ImportError:
        pass
    try:
        from trn_agent_boot.trn_boot import _ntff_profile_via_ctypes
        hook = _ntff_profile_via_ctypes('/opt/axon/libaxon_pjrt.so')
    except Exception:
        return
    mod = types.ModuleType('antenv.axon_hooks')
    mod.get_axon_ntff_profile_hook = lambda: hook
    mod.set_axon_ntff_profile_hook = lambda h: None
    sys.modules['antenv.axon_hooks'] = mod
    import antenv
    antenv.axon_hooks = mod


@functools.lru_cache(maxsize=1)
def _scores_prog():
    """Per core: scores[p, i] = tok[i*128 + p, :] . w  for 2048 tokens."""
    nc = bacc.Bacc("TRN2", target_bir_lowering=False, debug=False, num_devices=8)
    tok = nc.declare_dram_parameter("tok", [NHALF, D], mybir.dt.float32, isOutput=False)
    wbc = nc.declare_dram_parameter("wbc", [P, D], mybir.dt.float32, isOutput=False)
    sout = nc.declare_dram_parameter("scores", [P, NT1], mybir.dt.float32, isOutput=True)

    with tile.TileContext(nc) as tc:
        with (
            tc.tile_pool(name="sb", bufs=1) as sb,
            tc.tile_pool(name="toks", bufs=4) as toks,
            tc.tile_pool(name="prods", bufs=3) as prods,
        ):
            w_t = sb.tile([P, D], mybir.dt.float32)
            nc.sync.dma_start(out=w_t[:], in_=wbc[:])
            s_t = sb.tile([P, NT1], mybir.dt.float32)
            for i in range(NT1):
                tk = toks.tile([P, D], mybir.dt.float32, name=f"tk{i%4}", tag="tk")
                nc.sync.dma_start(out=tk[:], in_=tok[i * P:(i + 1) * P, :])
                pr = prods.tile([P, D], mybir.dt.float32, name=f"pr{i%3}", tag="pr")
                nc.vector.tensor_tensor(
                    out=pr[:], in0=tk[:], in1=w_t[:], op=mybir.AluOpType.mult,
                )
                nc.vector.tensor_reduce(
                    out=s_t[:, i:i + 1], in_=pr[:],
                    axis=mybir.AxisListType.X, op=mybir.AluOpType.add,
                )
            nc.sync.dma_start(out=sout[:], in_=s_t[:])
    nc.compile()
    return nc


@functools.lru_cache(maxsize=1)
def _attn_prog():
    """Per core: gather x=tok[idx] [C,D]; q,k,v for its 8 heads; attention over
    the C buffer; partial output rows (x @ .. @ wo_halfrows) * gate -> [C, D]."""
    nc = bacc.Bacc("TRN2", target_bir_lowering=False, debug=False, num_devices=8)
    f32, bf16, i32 = mybir.dt.float32, mybir.dt.bfloat16, mybir.dt.int32
    tok = nc.declare_dram_parameter("tok", [N, D], f32, isOutput=False)
    idx = nc.declare_dram_parameter("idx", [P, CT], i32, isOutput=False)
    gate = nc.declare_dram_parameter("gate", [P, CT], f32, isOutput=False)
    wq_d = nc.declare_dram_parameter("wq", [D, HHALF], bf16, isOutput=False)
    wk_d = nc.declare_dram_parameter("wk", [D, HHALF], bf16, isOutput=False)
    wv_d = nc.declare_dram_parameter("wv", [D, HHALF], bf16, isOutput=False)
    wo_d = nc.declare_dram_parameter("wo", [HHALF, D], bf16, isOutput=False)
    ident = nc.declare_dram_parameter("ident", [P, P], f32, isOutput=False)
    outp = nc.declare_dram_parameter("outp", [C, D], f32, isOutput=True)

    with tile.TileContext(nc) as tc:
        with (
            tc.tile_pool(name="sb", bufs=1) as sb,
            tc.tile_pool(name="xgp", bufs=3) as xgp,
            tc.tile_pool(name="etp", bufs=16) as etp,
            tc.tile_pool(name="osb", bufs=3) as osb,
            tc.tile_pool(name="ps_mm", bufs=3, space="PSUM") as ps_mm,
            tc.tile_pool(name="ps_att", bufs=2, space="PSUM") as ps_att,
            tc.tile_pool(name="ps_o", bufs=2, space="PSUM") as ps_o,
        ):
            # --- constants / small inputs ---
            id_t = sb.tile([P, P], f32)
            nc.sync.dma_start(out=id_t[:], in_=ident[:])
            ones64 = sb.tile([1, 64], bf16)
            nc.vector.memset(ones64[:], 1.0)
            idx_t = sb.tile([P, CT], i32)
            nc.sync.dma_start(out=idx_t[:], in_=idx[:])
            gate_t = sb.tile([P, CT], f32)
            nc.sync.dma_start(out=gate_t[:], in_=gate[:])
            # --- weights ---
            wq_t = [sb.tile([P, HHALF], bf16, name=f"wq{k}") for k in range(DT)]
            wk_t = [sb.tile([P, HHALF], bf16, name=f"wk{k}") for k in range(DT)]
            wv_t = [sb.tile([P, HHALF], bf16, name=f"wv{k}") for k in range(DT)]
            for k in range(DT):
                nc.sync.dma_start(out=wq_t[k][:], in_=wq_d[k * P:(k + 1) * P, :])
                nc.sync.dma_start(out=wk_t[k][:], in_=wk_d[k * P:(k + 1) * P, :])
                nc.sync.dma_start(out=wv_t[k][:], in_=wv_d[k * P:(k + 1) * P, :])
            wo_t = [sb.tile([P, D], bf16, name=f"wo{k}") for k in range(KT)]
            for k in range(KT):
                nc.sync.dma_start(out=wo_t[k][:], in_=wo_d[k * P:(k + 1) * P, :])

            # --- gather + transpose: xT[d-tile][d_in_tile, c] bf16 ---
            xT = [sb.tile([P, C], bf16, name=f"xT{k}") for k in range(DT)]
            for j in range(CT):
                xg = xgp.tile([P, D], f32, name=f"xg{j%3}", tag="xg")
                nc.gpsimd.indirect_dma_start(
                    out=xg[:], out_offset=None, in_=tok[:],
                    in_offset=bass.IndirectOffsetOnAxis(ap=idx_t[:, j:j + 1], axis=0),
                )
                for k in range(DT):
                    pt = ps_att.tile([P, P], f32, space="PSUM", name=f"pt{j%2}", tag="pa")
                    nc.tensor.transpose(out=pt[:], in_=xg[:, k * P:(k + 1) * P], identity=id_t[:])
                    nc.vector.tensor_copy(out=xT[k][:, j * P:(j + 1) * P], in_=pt[:])

            # --- qT, kT: [hd' (4 tiles), c] bf16 ---
            qT = [sb.tile([P, C], bf16, name=f"qT{m}") for m in range(KT)]
            kTt = [sb.tile([P, C], bf16, name=f"kT{m}") for m in range(KT)]
            for (w_t, dst) in ((wq_t, qT), (wk_t, kTt)):
                for m in range(KT):
                    for nch in range(2):
                        pm = ps_mm.tile([P, 512], f32, space="PSUM", name="pmm", tag="pmm")
                        for k in range(DT):
                            nc.tensor.matmul(
                                out=pm[:],
                                lhsT=w_t[k][:, m * P:(m + 1) * P],
                                rhs=xT[k][:, nch * 512:(nch + 1) * 512],
                                start=(k == 0), stop=(k == DT - 1),
                            )
                        nc.vector.tensor_copy(out=dst[m][:, nch * 512:(nch + 1) * 512], in_=pm[:])

            # --- v_aug: [c (8 tiles), 8 heads x (64 v-cols + ones)] bf16 ---
            va = [sb.tile([P, 8, 65], bf16, name=f"va{mt}") for mt in range(CT)]
            for mt in range(CT):
                pm = ps_mm.tile([P, 512], f32, space="PSUM", name="pmm", tag="pmm")
                for k in range(DT):
                    nc.tensor.matmul(
                        out=pm[:],
                        lhsT=xT[k][:, mt * P:(mt + 1) * P],
                        rhs=wv_t[k][:],
                        start=(k == 0), stop=(k == DT - 1),
                    )
                pm3 = pm[:].rearrange("p (h c) -> p h c", h=8)
                nc.vector.tensor_copy(out=va[mt][:, :, 0:64], in_=pm3[:])
                nc.vector.memset(va[mt][:, :, 64:65], 1.0)

            # --- attention: per (head l, q-chunk qc) ---
            # attT[k, q] = (k_l q_l^T); eT = exp(attT/8) bf16; oT[65, 512] = va_l^T @ eT
            # row 64 of oT is Z (softmax denominator); oss = oT[0:64] / Z.
            oss = [sb.tile([P, C], bf16, name=f"oss{m}") for m in range(KT)]
            for l in range(8):
                m4, p64 = l // 2, (l % 2) * 64
                for qc in range(2):
                    eT = []
                    for mt in range(CT):
                        pa = ps_att.tile([P, 512], f32, space="PSUM", name=f"pa{mt%2}", tag="pa")
                        nc.tensor.matmul(
                            out=pa[:],
                            lhsT=kTt[m4][p64:p64 + 64, mt * P:(mt + 1) * P],
                            rhs=qT[m4][p64:p64 + 64, qc * 512:(qc + 1) * 512],
                            start=True, stop=True,
                        )
                        et = etp.tile([P, 512], bf16, name=f"eT{mt}", tag=f"eT{mt}", bufs=2)
                        nc.scalar.activation(out=et[:], in_=pa[:],
                                             func=mybir.ActivationFunctionType.Exp, scale=0.125)
                        eT.append(et)
                    po = ps_o.tile([65, 512], f32, space="PSUM", name="po", tag="po")
                    for mt in range(CT):
                        nc.tensor.matmul(
                            out=po[:],
                            lhsT=va[mt][:, l, :],
                            rhs=eT[mt][:],
                            start=(mt == 0), stop=(mt == CT - 1),
                        )
                    # Z -> broadcast to 64 partitions -> 1/Z -> scale oT rows
                    zqb = etp.tile([1, 512], bf16, name="zqb", tag="zqb", bufs=2)
                    nc.vector.tensor_copy(out=zqb[:], in_=po[64:65, :])
                    pz = ps_mm.tile([64, 512], f32, space="PSUM", name="pz", tag="pmm")
                    nc.tensor.matmul(out=pz[:], lhsT=ones64[:], rhs=zqb[:],
                                     start=True, stop=True)
                    rb = etp.tile([64, 512], f32, name="rb", tag="rb", bufs=2)
                    nc.vector.reciprocal(out=rb[:], in_=pz[:])
                    nc.vector.tensor_tensor(
                        out=oss[m4][p64:p64 + 64, qc * 512:(qc + 1) * 512],
                        in0=po[0:64, :],
                        in1=rb[:],
                        op=mybir.AluOpType.mult,
                    )

            # --- output projection + gate ---
            for mt in range(CT):
                ot = osb.tile([P, D], f32, name=f"ot{mt%3}", tag="ot")
                for nch in range(2):
                    pm = ps_mm.tile([P, 512], f32, space="PSUM", name="pmm", tag="pmm")
                    for k in range(KT):
                        nc.tensor.matmul(
                            out=pm[:],
                            lhsT=oss[k][:, mt * P:(mt + 1) * P],
                            rhs=wo_t[k][:, nch * 512:(nch + 1) * 512],
                            start=(k == 0), stop=(k == KT - 1),
                        )
                    nc.vector.tensor_scalar_mul(
                        out=ot[:, nch * 512:(nch + 1) * 512],
                        in0=pm[:], scalar1=gate_t[:, mt:mt + 1],
                    )
                nc.sync.dma_start(out=outp[mt * P:(mt + 1) * P, :], in_=ot[:])
    nc.compile()
    return nc


def _run(nc, in_maps, trace=False):
    _install_ntff_hook()
    return run_bass_kernel_spmd(nc, in_maps, core_ids=list(range(8)), trace=trace)


_last_exec_ns = {}


def kernel(token_inputs, w_router, wq, wk, wv, wo, _trace=False):
    token_inputs = np.ascontiguousarray(np.asarray(token_inputs, np.float32))
    w_router = np.asarray(w_router, np.float32)

    # ---- launch 1: router scores ----
    wbc = np.ascontiguousarray(np.broadcast_to(w_router.reshape(1, D), (P, D)))
    maps1 = []
    for c in range(8):
        b, hf = c // 2, c % 2
        maps1.append({
            "tok": np.ascontiguousarray(token_inputs[b, hf * NHALF:(hf + 1) * NHALF, :]),
            "wbc": wbc,
        })
    r1 = _run(_scores_prog(), maps1, trace=_trace)
    _last_exec_ns['scores'] = r1.exec_time_ns
    scores = np.empty((B, N), np.float32)
    for c in range(8):
        b, hf = c // 2, c % 2
        # scores tile layout: [p, i] = token i*128+p
        scores[b, hf * NHALF:(hf + 1) * NHALF] = r1.results[c]["scores"].T.reshape(-1)

    z_loss = np.float32(np.mean(scores.astype(np.float64) ** 2))

    # ---- host: top-C selection, gates ----
    sel = np.empty((B, C), np.int64)
    for b in range(B):
        top = np.argpartition(-scores[b], C - 1)[:C]
        sel[b] = np.sort(top)
    gates = 1.0 / (1.0 + np.exp(-scores[np.arange(B)[:, None], sel].astype(np.float64)))
    gates = gates.astype(np.float32)

    # ---- launch 2: attention over the capacity buffer ----
    ident = np.eye(P, dtype=np.float32)
    bdiag = np.zeros((8, HHALF), BF16)
    for l in range(8):
        bdiag[l, l * 64:(l + 1) * 64] = 1.0
    maps2 = []
    for c in range(8):
        b, hh = c // 2, c % 2
        maps2.append({
            "tok": token_inputs[b],
            "idx": np.ascontiguousarray(sel[b].astype(np.int32).reshape(CT, P).T),
            "gate": np.ascontiguousarray(gates[b].reshape(CT, P).T),
            "wq": np.ascontiguousarray(np.asarray(wq, np.float32)[:, hh * HHALF:(hh + 1) * HHALF]).astype(BF16),
            "wk": np.ascontiguousarray(np.asarray(wk, np.float32)[:, hh * HHALF:(hh + 1) * HHALF]).astype(BF16),
            "wv": np.ascontiguousarray(np.asarray(wv, np.float32)[:, hh * HHALF:(hh + 1) * HHALF]).astype(BF16),
            "wo": np.ascontiguousarray(np.asarray(wo, np.float32)[hh * HHALF:(hh + 1) * HHALF, :]).astype(BF16),
            "ident": ident,
            "bdiag": bdiag,
        })
    r2 = _run(_attn_prog(), maps2, trace=_trace)
    _last_exec_ns['attn'] = r2.exec_time_ns

    out = np.zeros((B, N, D), np.float32)
    for b in range(B):
        out[b, sel[b], :] = r2.results[2 * b]["outp"] + r2.results[2 * b + 1]["outp"]
    return out, z_loss
